# revision 60
# baseline (speedup 1.0000x reference)
"""Linear-chain CRF negative mean log-likelihood on 8 Trainium2 NeuronCores.

Full inputs in, full (scalar) output out. Data-parallel over the batch: each
core processes B/8 = 1024 sequences end-to-end.

v3 architecture (per core), engine-balanced around the serial forward-DP
(~650ns/step steady state, PE-bound):
  - emission scores em[32g+l, b'] per step via 4 matmuls: group 0 rides the
    fp8 DoubleRow perf mode (0.5 cy/col; only legal at PSUM partition 0),
    groups 1-3 plain fp8. x is host-marshalled into the matching layouts.
  - partition function via the exp-space forward DP
    A_t = (expBD^T A_{t-1}) o exp(em_t - c_t), split into two 128-column
    half-chains; the second DP half is placed after two em matmuls so the
    DVE-serialized second A-multiply is never the critical edge. The DVE
    runs ONLY these two A-multiplies per step.
  - the PE instruction stream is total-ordered in program order (explicit
    add_dep_helper chain): the greedy Tile scheduler otherwise interleaves
    the serial DP matmuls behind a step's worth of em/gather work.
  - gold emission score: Pool computes P_t = OHT_t o E_t (the only
    PSUM-free elementwise engine, ~600ns); a ones-gather matmul with a
    slot-shifted stationary accumulates s[4s+g, b'] = E_t[y_t, b] into a
    rotating PSUM block (8 steps per block). P is issued one iteration
    ahead and the gather consumes it one iteration later, so the Pool op
    always has >1 period of slack before its in-order PE consumer. Act
    copies blocks out; the host does the ln + sum (the c-schedule cancels
    against logZ).
  - gold transition score: paired count matmuls (even anchors, moving
    [oh_{t-1} | oh_{t+1}], fp8 DoubleRow, 26 cy each) accumulate
    [l_t, l_prev | l_next] counts, 4 matmuls/step split across even/odd
    steps; host contracts with Tr^T | Tr.
  - logZ: group sums zs = onesBD^T A_63 shipped raw; host ln + reduce.
  - 5 warmup matmuls hold the PE p-state ramp; DMAs stream in
    progressively larger chunks ordered by first use (every consumer of a
    DMA pays a 900ns semaphore-propagation penalty, so chunks are sized to
    land several steps ahead).

Each core writes partial tensors; the host combines them into the loss.
"""

import numpy as np

L = 26
D = 128
T = 64
B = 8192
NCORES = 8
BC = B // NCORES  # 1024 sequences per core

# Per-step scale schedule for the exp-space forward DP (subtracted from em at
# step t so the running A stays well inside fp32 range). It cancels exactly
# in the host finale (gold ln-sum and logZ shift by the same B*sum(C_SCHED)).
C_SCHED = np.array([
    0.933700, 3.577268, 3.746262, 4.537820, 4.040299, 4.041378, 4.067604, 4.107736,
    4.101158, 4.091968, 3.790887, 4.203616, 4.050755, 4.272369, 3.625527, 3.864683,
    4.922722, 4.424649, 3.161501, 4.352942, 3.777887, 4.534618, 4.044740, 3.829787,
    4.015547, 4.710327, 3.921810, 4.398400, 4.176108, 3.293104, 4.761852, 3.388780,
    3.782803, 4.950686, 3.611373, 4.506680, 3.005395, 4.511179, 3.714007, 4.567758,
    3.993558, 4.003791, 4.249708, 4.211322, 4.069564, 4.249093, 3.763951, 3.601156,
    5.005219, 3.880518, 4.270474, 3.819207, 3.979380, 4.438228, 4.122883, 2.404448,
    4.026374, 5.060853, 4.290274, 4.044138, 3.681486, 4.656340, 3.408876, 3.532320,
], dtype=np.float64)

_CACHE: dict = {}
TRACE = False  # set by test harness to capture NTFF profile / exec time

# Instruction opcodes whose hardware structs tolerate multiple sync waits (or
# that walrus lowers specially). Everything else gets excess waits peeled onto
# EventSemaphore instructions inserted just before it (same engine).
_MULTIWAIT_OK = {
    "Call",
    "UnconditionalBranch",
    "ConditionalBranch",
}


def _legalize_waits(bir_bytes: bytes) -> bytes:
    """Split >1 sync waits per compute instruction into EventSemaphore preludes.

    The TRN2 64-byte instruction structs hold a single sync-wait command;
    Tile attaches multi-engine waits directly, which walrus codegen rejects
    ("Too many sync wait commands"). Peeling extra waits onto same-engine
    EventSemaphore instructions placed immediately before is semantically
    identical (engine streams execute in order).
    """
    import json

    d = json.loads(bir_bytes)
    n = 0
    for fn in d["functions"]:
        for blk in fn["blocks"]:
            out = []
            for inst in blk["instructions"]:
                si = inst.get("sync_info")
                if (
                    si
                    and len(si.get("on_wait", [])) > 1
                    and inst["opcode"] not in _MULTIWAIT_OK
                ):
                    waits = si["on_wait"]
                    for w in waits[:-1]:
                        n += 1
                        out.append({
                            "debug": inst.get("debug", 0),
                            "engine": inst["engine"],
                            "ins": [],
                            "name": f"wsplit-{n}-{inst['name']}",
                            "opcode": "EventSemaphore",
                            "outs": [],
                            "sync_info": {"on_update": [], "on_wait": [w]},
                        })
                    si["on_wait"] = [waits[-1]]
                out.append(inst)
            blk["instructions"] = out
    return json.dumps(d).encode()


# cblob byte layout (per partition)
_CB_WDR = 0        # [0:64)    Wdr fp8 [64 part, 2, 32]  DoubleRow weights
_CB_WT = 64        # [64:96)   Wt32 fp8 [128, 32]        plain em weights (W^T)
_CB_EXPBD = 96     # [96:352)  expBD bf16 [128, 128]     block-diag exp(Tr)
_CB_CBIAS = 352    # [352:608) cbias f32 [128, 64]       -C_SCHED broadcast
_CB_ONESSH = 608   # [608:1120) onesSh bf16 [128, 8, 32] slot-shifted gather
_CB_ONESBD = 1120  # [1120:1128) onesBD bf16 [128, 4]    group-sum mask
_CB_END = 1128

# out tensor column layout (f32 [128, 2368])
_OUT_S = 0         # rows 0:32, cols [0:2048)   s blocks [32, 8, 256]
_OUT_A = 2048      # rows 0:128, cols [2048:2176) A_63 bf16 [128, 256] bitcast
_OUT_CC = 2176     # rows 0:26, cols [2176:2228) CC [26, 52]
_OUT_COLS = 2368


def build_program():
    """Build the per-core Bass/Tile program (identical SPMD program)."""
    from contextlib import ExitStack

    import concourse.bass as bass
    import concourse.tile as tile
    from concourse import mybir

    f32 = mybir.dt.float32
    bf16 = mybir.dt.bfloat16
    f8 = mybir.dt.float8e4
    AF = mybir.ActivationFunctionType
    OP = mybir.AluOpType
    DR = mybir.MatmulPerfMode.DoubleRow

    nc = bass.Bass("TRN2", target_bir_lowering=False, debug=False)

    xa_d = nc.dram_tensor("xa", [64, 2, T, 256], f8, kind="ExternalInput").ap()
    xb_d = nc.dram_tensor("xb", [D, T, 768], f8, kind="ExternalInput").ap()
    oht_d = nc.dram_tensor("oht", [128, T, 256], f8, kind="ExternalInput").ap()
    ohdr_d = nc.dram_tensor("ohdr", [64, 2, T, 8, L], f8, kind="ExternalInput").ap()
    c_d = nc.dram_tensor("cst", [128, _CB_END], mybir.dt.uint8, kind="ExternalInput").ap()
    out_d = nc.dram_tensor("out", [128, _OUT_COLS], f32, kind="ExternalOutput").ap()

    from concourse.tile import add_dep_helper

    # Total-order the PE instruction stream in program order: the greedy Tile
    # scheduler otherwise slots em/gather matmuls ahead of the next step's DP
    # matmul whenever the DP's input isn't ready yet in its internal sim,
    # which threads the serial DP->DVE chain through a step's worth of PE
    # work (in-order engine streams) and inflates the critical cycle.
    _pe_prev = [None]

    def pe_mm(*args, **kwargs):
        mi = nc.tensor.matmul(*args, **kwargs)
        if _pe_prev[0] is not None:
            add_dep_helper(mi.ins, _pe_prev[0].ins, reason="pe-order")
        _pe_prev[0] = mi
        return mi

    with ExitStack() as ctx:
        tc = ctx.enter_context(tile.TileContext(nc))

        const = ctx.enter_context(tc.tile_pool(name="const", bufs=1))
        epool = ctx.enter_context(tc.tile_pool(name="epool", bufs=8))
        apool = ctx.enter_context(tc.tile_pool(name="apool", bufs=2))
        ppool = ctx.enter_context(tc.tile_pool(name="ppool", bufs=5))
        fpool = ctx.enter_context(tc.tile_pool(name="fpool", bufs=1))
        ps_em = ctx.enter_context(tc.tile_pool(name="ps_em", bufs=2, space="PSUM"))
        ps_u1 = ctx.enter_context(tc.tile_pool(name="ps_u1", bufs=1, space="PSUM"))
        ps_u2 = ctx.enter_context(tc.tile_pool(name="ps_u2", bufs=1, space="PSUM"))
        ps_sg = ctx.enter_context(tc.tile_pool(name="ps_sg", bufs=2, space="PSUM"))
        ps_cc = ctx.enter_context(tc.tile_pool(name="ps_cc", bufs=1, space="PSUM"))

        # ---- PE p-state warmup: dummy matmuls keep the tensor engine's
        # ramp running so the first real emissions hit full clock
        wz = const.tile([128, 256], bf16)
        nc.vector.memset(wz, 0.0)
        for w in range(5):
            wps = ps_em.tile([128, 256], f32, tag="em", name="warm")
            pe_mm(
                wps, lhsT=wz[:, 0:128], rhs=wz[:, 0:256], start=True, stop=True
            )

        # ---- SBUF input tiles ----
        xa = const.tile([64, 2, T, 256], f8)
        xb = const.tile([D, T, 768], f8)
        oht = const.tile([128, T, 256], f8)
        ohdr = const.tile([64, 2, T, 8, L], f8)
        cblob = const.tile([128, _CB_END], mybir.dt.uint8)
        fin = fpool.tile([128, _OUT_COLS], f32)

        # packed constants first: single small DMA gates everything
        nc.scalar.dma_start(out=cblob, in_=c_d)

        def dma_xa(t0, t1):
            nc.sync.dma_start(out=xa[:, :, t0:t1, :], in_=xa_d[:, :, t0:t1, :])

        def dma_xb(t0, t1):
            nc.sync.dma_start(out=xb[:, t0:t1, :], in_=xb_d[:, t0:t1, :])

        def dma_oht(t0, t1):
            nc.sync.dma_start(out=oht[:, t0:t1, :], in_=oht_d[:, t0:t1, :])

        def dma_ohdr(t0, t1):
            nc.sync.dma_start(
                out=ohdr[:, :, t0:t1, :, :], in_=ohdr_d[:, :, t0:t1, :, :]
            )

        # front-load tiny first chunks (em(0)/em(1) gate the chain start,
        # and every consumer pays the 900ns DMA-sem propagation), then
        # stream progressively larger chunks ordered several steps ahead
        # of first use; x (chain-critical) leads, oht/ohdr interleave
        # fine-grained stream interleaved in need-order (x[t] consumed at
        # iteration t-2, oht[t] at t-1 via P, ohdr[t] at ~t+1 via counts),
        # with x leading by a block so late chunks never stall the chain
        dma_xa(0, 2)
        dma_xb(0, 2)
        dma_xa(2, 6)
        dma_xb(2, 6)
        dma_oht(0, 4)
        dma_xa(6, 10)
        dma_xb(6, 10)
        dma_oht(4, 8)
        dma_ohdr(0, 6)
        dma_xa(10, 14)
        dma_xb(10, 14)
        dma_oht(8, 12)
        dma_ohdr(6, 10)
        dma_xa(14, 18)
        dma_xb(14, 18)
        dma_oht(12, 16)
        dma_ohdr(10, 14)
        dma_xa(18, 22)
        dma_xb(18, 22)
        dma_oht(16, 20)
        dma_ohdr(14, 18)
        dma_xa(22, 28)
        dma_xb(22, 28)
        dma_oht(20, 26)
        dma_ohdr(18, 24)
        dma_xa(28, 36)
        dma_xb(28, 36)
        dma_oht(26, 34)
        dma_ohdr(24, 32)
        dma_xa(36, 46)
        dma_xb(36, 46)
        dma_oht(34, 44)
        dma_ohdr(32, 42)
        dma_xa(46, 56)
        dma_xb(46, 56)
        dma_oht(44, 54)
        dma_ohdr(42, 52)
        dma_xa(56, 64)
        dma_xb(56, 64)
        dma_oht(54, 64)
        dma_ohdr(52, 64)

        # ---- bitcast views into the packed constant blob ----
        Wdr = cblob[0:64, _CB_WDR : _CB_WDR + 64].bitcast(f8).rearrange(
            "p (j m) -> p j m", j=2
        )
        Wt32 = cblob[:, _CB_WT : _CB_WT + 32].bitcast(f8)
        expBD = cblob[:, _CB_EXPBD : _CB_EXPBD + 256].bitcast(bf16)
        cbias = cblob[:, _CB_CBIAS : _CB_CBIAS + 256].bitcast(f32)
        onesSh = cblob[:, _CB_ONESSH : _CB_ONESSH + 512].bitcast(bf16).rearrange(
            "p (s m) -> p s m", s=8
        )
        onesBD = cblob[:, _CB_ONESBD : _CB_ONESBD + 8].bitcast(bf16)

        # persistent psum accumulator for paired transition counts
        CC_ps = ps_cc.tile([L, 2 * L], f32)
        nc.vector.memset(CC_ps, 0.0)

        E_t = {}
        em_t = {}

        def emit_em_a(t):
            # group 0 (fp8 DoubleRow; only legal at psum partition 0) and
            # groups 1-2 plain fp8
            em_ps = ps_em.tile([128, 256], f32, tag="em")
            em_t[t] = em_ps
            pe_mm(
                em_ps[0:32, :],
                lhsT=Wdr,
                rhs=xa[:, :, t, :],
                start=True,
                stop=True,
                perf_mode=DR,
                tile_position=(0, 0),
            )
            for g in (1, 2):
                pe_mm(
                    em_ps[32 * g : 32 * (g + 1), :],
                    lhsT=Wt32,
                    rhs=xb[:, t, 256 * (g - 1) : 256 * g],
                    start=True,
                    stop=True,
                    tile_position=(0, 32 * g),
                )

        def emit_em_b(t):
            # group 3 plain fp8 (placed after the step's second DP half)
            pe_mm(
                em_t[t][96:128, :],
                lhsT=Wt32,
                rhs=xb[:, t, 512:768],
                start=True,
                stop=True,
                tile_position=(0, 96),
            )

        def emit_em(t):
            emit_em_a(t)
            emit_em_b(t)

        def emit_exp(t):
            E = epool.tile([128, 256], bf16, tag="E", name="E")
            nc.scalar.activation(
                E, em_t.pop(t), AF.Exp, bias=cbias[:, t : t + 1], scale=1.0
            )
            E_t[t] = E

        sg_tiles = {}

        P_t = {}

        def emit_P(t):
            # P_t = OHT_t o E_t on Pool (the only PSUM-free elementwise
            # engine); issued one step ahead of the gather so the ~600ns
            # Pool op stays off the PE queue's critical path
            P = ppool.tile([128, 256], bf16, tag="P", name="P")
            nc.gpsimd.tensor_tensor(out=P, in0=oht[:, t, :], in1=E_t[t], op=OP.mult)
            P_t[t] = P

        def emit_gather(t):
            # slot-shifted ones-gather accumulates E_t[y_t, b] into psum
            s, q = t % 8, t // 8
            if s == 0:
                sg_tiles[q % 2] = ps_sg.tile([32, 256], f32, tag="sg", name="sg")
            pe_mm(
                sg_tiles[q % 2],
                lhsT=onesSh[:, s, :],
                rhs=P_t.pop(t),
                start=(s == 0),
                stop=(s == 7),
            )

        def emit_scopy(q):
            nc.scalar.copy(fin[0:32, 256 * q : 256 * (q + 1)], sg_tiles[q % 2])
            if q % 2 == 1 and q < 7:
                nc.sync.dma_start(
                    out=out_d[0:32, 256 * (q - 1) : 256 * (q + 1)],
                    in_=fin[0:32, 256 * (q - 1) : 256 * (q + 1)],
                )

        def emit_counts(a, chunks=range(8)):
            # paired transition counts, anchor a (even): one fp8 DoubleRow
            # matmul per b-chunk covers pairs (a-1,a) [transposed] and (a,a+1)
            for c in chunks:
                lhsT = ohdr[:, :, a, c, :]
                if a == 0:
                    rhs = ohdr[:, :, 1:2, c, :]
                    outap = CC_ps[:, L : 2 * L]
                else:
                    rhs = ohdr[:, :, a - 1 : a + 2 : 2, c, :]
                    outap = CC_ps
                pe_mm(
                    outap,
                    lhsT=lhsT,
                    rhs=rhs,
                    start=False,
                    stop=False,
                    perf_mode=DR,
                    skip_group_check=True,
                )

        # ---- software-pipelined main loop ----
        # bank six steps of E up front: the early DMA-pipeline deficit is
        # then paid once, in the prologue, instead of repeatedly mid-loop
        # (each mid-loop stall costs a fresh 900ns DMA-sem propagation)
        LOOKAHEAD = 2
        for t0 in range(LOOKAHEAD + 1):
            emit_em(t0)
            emit_exp(t0)
        emit_P(0)
        A_prev = None
        for t in range(T):
            E = E_t[t]
            if t == 0:
                A_prev = E
            else:
                # chain halves: DP-H2 is deliberately placed after two em
                # matmuls so its consumer (the DVE-serialized second A-half)
                # is never the critical edge
                with tc.high_priority(offset=60):
                    u1 = ps_u1.tile([128, 128], f32, tag="u1", name="u1")
                    if t == T - 1:
                        # final A lands straight in the staging tile: the
                        # host derives the logZ group-sums from raw A_63,
                        # skipping the zs matmul + copy in the tail
                        A_new = fin[0:128, _OUT_A : _OUT_A + 128].bitcast(bf16)
                    else:
                        A_new = apool.tile([128, 256], bf16, tag="A", name="A")
                    pe_mm(u1, lhsT=expBD, rhs=A_prev[:, 0:128], start=True, stop=True)
                    nc.vector.tensor_mul(A_new[:, 0:128], u1, E[:, 0:128])
                if t + LOOKAHEAD < T:
                    emit_em_a(t + LOOKAHEAD)
                with tc.high_priority(offset=60):
                    u2 = ps_u2.tile([128, 128], f32, tag="u2", name="u2")
                    pe_mm(u2, lhsT=expBD, rhs=A_prev[:, 128:256], start=True, stop=True)
                    nc.vector.tensor_mul(A_new[:, 128:256], u2, E[:, 128:256])
                if t + LOOKAHEAD < T:
                    emit_em_b(t + LOOKAHEAD)
                    emit_exp(t + LOOKAHEAD)
                A_prev = A_new
            if t + 1 < T:
                emit_P(t + 1)
            if t >= 2:
                # gather runs two iterations behind its P: the ~600ns Pool
                # op gets >2 periods of slack before its in-order PE
                # consumer, and the oht DMA stream can trail the
                # chain-critical x stream by two extra steps
                emit_gather(t - 2)
                if (t - 2) % 8 == 7:
                    emit_scopy((t - 2) // 8)
            E_t.pop(t)
            if t >= 2 and t % 2 == 0:
                emit_counts(t - 2, range(4))
            elif t >= 3:
                emit_counts(t - 3, range(4, 8))
        emit_gather(T - 2)
        emit_gather(T - 1)
        emit_scopy(7)
        emit_counts(T - 2)

        # ---- finale: A_63 is already staged in fin; ship A+CC first
        # (ready earliest), s blocks 6-7 after scopy(7) ----
        nc.vector.tensor_copy(fin[0:L, _OUT_CC : _OUT_CC + 2 * L], CC_ps)
        nc.sync.dma_start(
            out=out_d[0:128, _OUT_A : _OUT_CC + 2 * L],
            in_=fin[0:128, _OUT_A : _OUT_CC + 2 * L],
        )
        nc.sync.dma_start(out=out_d[0:32, 1536:2048], in_=fin[0:32, 1536:2048])

    fixed = _legalize_waits(nc.to_json_bytes())
    nc.to_json_bytes = lambda: fixed  # shadow for all compile paths
    return nc


def _marshal(feat_x, input_y, params):
    """Host-side input marshalling: dtype casts + layout transposes/onehots."""
    import ml_dtypes

    f8 = ml_dtypes.float8_e4m3
    bf16 = ml_dtypes.bfloat16

    feat_x = np.asarray(feat_x, dtype=np.float32)
    input_y = np.asarray(input_y, dtype=np.int32)
    params = np.asarray(params, dtype=np.float32)

    W = params[: L * D].reshape(L, D)
    Tr = params[L * D :].reshape(L, L).astype(np.float64)

    # ---- packed per-partition constants ----
    cblob = np.zeros((128, _CB_END), dtype=np.uint8)
    # Wdr [64, 2, 32]: Wdr[k, j, m] = W[m, 2k+j]
    wdr = np.zeros((64, 2, 32), dtype=np.float32)
    wdr[:, :, :L] = W.T.reshape(64, 2, L)
    cblob[0:64, _CB_WDR : _CB_WDR + 64] = (
        wdr.astype(f8).view(np.uint8).reshape(64, 64)
    )
    # Wt32 [128, 32]: W^T zero-padded
    wt32 = np.zeros((D, 32), dtype=np.float32)
    wt32[:, :L] = W.T
    cblob[:, _CB_WT : _CB_WT + 32] = wt32.astype(f8).view(np.uint8)
    # expBD block-diag exp(Tr)
    expbd = np.zeros((128, 128), dtype=np.float32)
    for g in range(4):
        expbd[32 * g : 32 * g + L, 32 * g : 32 * g + L] = np.exp(Tr)
    cblob[:, _CB_EXPBD : _CB_EXPBD + 256] = expbd.astype(bf16).view(np.uint8)
    # cbias
    cbias = np.tile(-C_SCHED.astype(np.float32), (128, 1))
    cblob[:, _CB_CBIAS : _CB_CBIAS + 256] = cbias.view(np.uint8)
    # onesSh [128, 8, 32]: onesSh[32g+l, s, 4s+g] = 1 for l < L
    onessh = np.zeros((128, 8, 32), dtype=np.float32)
    for g in range(4):
        for s in range(8):
            onessh[32 * g : 32 * g + L, s, 4 * s + g] = 1.0
    cblob[:, _CB_ONESSH : _CB_ONESSH + 512] = (
        onessh.astype(bf16).view(np.uint8).reshape(128, 512)
    )
    # onesBD [128, 4]
    onesbd = np.zeros((128, 4), dtype=np.float32)
    for g in range(4):
        onesbd[32 * g : 32 * g + L, g] = 1.0
    cblob[:, _CB_ONESBD : _CB_ONESBD + 8] = onesbd.astype(bf16).view(np.uint8)
    cblob = np.ascontiguousarray(cblob)

    # x transposed once: xT[d, t, b]
    xT = np.ascontiguousarray(feat_x.transpose(2, 1, 0)).astype(f8)

    in_maps = []
    for m in range(NCORES):
        sl = slice(m * BC, (m + 1) * BC)
        xm = xT[:, :, sl]  # [128, T, 1024] fp8
        ym = input_y[sl]  # [1024, T]
        # xa [64, 2, T, 256]: group 0 (b 0:256), d = 2k+j
        xam = np.ascontiguousarray(
            xm[:, :, 0:256].reshape(64, 2, T, 256)
        )
        # xb [128, T, 768]: groups 1-3 (b 256:1024)
        xbm = np.ascontiguousarray(xm[:, :, 256:1024])
        # oht [128, T, 256]: 448 where (y[256g+b', t] == l) else 0 - the
        # device masks via min(oht, E) on the Pool engine
        lab = ym.reshape(4, 256, T)  # [g, b', t]
        lvec = np.arange(32)
        ohtm = (
            lab[:, None, :, :] == lvec[None, :, None, None]
        )  # [g, l(32), b', t]
        ohtm = np.ascontiguousarray(
            ohtm.reshape(128, 256, T).transpose(0, 2, 1).astype(np.float32)
        ).astype(f8)
        # ohdr [64, 2, T, 8, L]: ohdr[k, j, t, c, l] = (y[128c+2k+j, t] == l)
        labc = ym.reshape(8, 64, 2, T)  # [c, k, j, t]
        ohdrm = labc[:, :, :, :, None] == np.arange(L)[None, None, None, None, :]
        ohdrm = np.ascontiguousarray(
            ohdrm.transpose(1, 2, 3, 0, 4).astype(np.float32)
        ).astype(f8)
        in_maps.append(
            {"xa": xam, "xb": xbm, "oht": ohtm, "ohdr": ohdrm, "cst": cblob}
        )
    return in_maps


def kernel(feat_x: np.ndarray, input_y: np.ndarray, params: np.ndarray) -> np.ndarray:
    from concourse.bass_utils import run_bass_kernel_spmd

    if "nc" not in _CACHE:
        _CACHE["nc"] = build_program()
    nc = _CACHE["nc"]

    in_maps = _marshal(feat_x, input_y, params)

    try:
        res = run_bass_kernel_spmd(
            nc, in_maps, core_ids=list(range(NCORES)), trace=TRACE
        )
    except ModuleNotFoundError:
        # no NTFF profiling hook in this environment (antenv.axon_hooks);
        # rerun without tracing
        res = run_bass_kernel_spmd(
            nc, in_maps, core_ids=list(range(NCORES)), trace=False
        )
    _CACHE["last_results"] = res

    params = np.asarray(params, dtype=np.float64)
    Tr = params[L * D :].reshape(L, L)

    import ml_dtypes

    lns_sum = lnz_sum = tr_sum = 0.0
    for m in range(NCORES):
        out = res.results[m]["out"]
        s = out[0:32, _OUT_S : _OUT_S + 2048].astype(np.float64)
        lns_sum += np.log(np.maximum(s, 1e-300)).sum()
        # the f32 columns hold A_63's bf16 pairs; reinterpret the bytes
        a63 = (
            np.ascontiguousarray(out[0:128, _OUT_A : _OUT_A + 128])
            .view(ml_dtypes.bfloat16)
            .astype(np.float64)
        )
        zs = a63.reshape(4, 32, 256)[:, :L, :].sum(axis=1)
        lnz_sum += np.log(zs).sum()
        cc = out[0:L, _OUT_CC : _OUT_CC + 2 * L].astype(np.float64)
        tr_sum += (Tr.T * cc[:, 0:L]).sum() + (Tr * cc[:, L : 2 * L]).sum()
    loss = -(lns_sum + tr_sum - lnz_sum) / B
    return np.float32(loss)
